# revision 1
# baseline (speedup 1.0000x reference)
"""GPT (4-layer, B=4 T=1024 C=1024 NH=8) Trainium2 Bass kernel.

Sharding: 8 cores = 4 batches (DP) x 2 sequence halves (SP).
Per layer each core computes LN1, Q/K/V projections for its own 512
tokens, AllGathers K,V (bf16) with its pair core, runs dense attention
for its 512 query tokens against all 1024 kv tokens (causal + column
mask + graph pbias applied via per-core data), then proj, LN2, MLP.
Final LN + EinLinear head per 64-residue class.

Everything on-chip is feature-major (C on partitions, tokens on free
axis) so all matmuls chain with weights used as stored. Matmul operands
bf16, accumulation fp32, residual/softmax/LN fp32.
"""

import os
import sys

import numpy as np

for _p in ("/opt/trn_rl_repo",):
    if _p not in sys.path and os.path.isdir(_p):
        sys.path.insert(0, _p)

import ml_dtypes  # noqa: E402

BF16 = ml_dtypes.bfloat16

# model dims
B, T, C, L = 4, 1024, 1024, 4
NH, HD = 8, 128
HID = 4 * C
V1 = 101  # V + 1
TD = 64  # TOTAL_DIM
NUM_NODE, F_DIM = 15, 4
D_BIAS = NUM_NODE * F_DIM  # 60
EPS = 1e-5
NCORES = 8
TOK = T // 2  # 512 tokens per core
KSUB = C // 128  # 8
HSUB = HID // 128  # 32

_CACHED = {}


def _build_program():
    """Build the Bass SPMD program (same program for all 8 cores)."""
    import concourse.bacc as bacc
    import concourse.bass as bass  # noqa: F401
    import concourse.mybir as mybir
    import concourse.tile as tile

    dt = mybir.dt
    AF = mybir.ActivationFunctionType
    OP = mybir.AluOpType

    nc = bacc.Bacc(trn_type="TRN2", num_devices=NCORES)

    # ---- I/O ----
    h0T_d = nc.dram_tensor("h0T", (C, TOK), dt.float32, kind="ExternalInput")
    maskT_d = nc.dram_tensor("maskT", (KSUB, 128, TOK), dt.bfloat16, kind="ExternalInput")
    pbT_d = nc.dram_tensor("pbT", (64, NH, 64), dt.float32, kind="ExternalInput")
    wq_d = nc.dram_tensor("wq", (L, C, C), dt.bfloat16, kind="ExternalInput")
    wk_d = nc.dram_tensor("wk", (L, C, C), dt.bfloat16, kind="ExternalInput")
    wv_d = nc.dram_tensor("wv", (L, C, C), dt.bfloat16, kind="ExternalInput")
    wp_d = nc.dram_tensor("wp", (L, C, C), dt.bfloat16, kind="ExternalInput")
    w1_d = nc.dram_tensor("w1", (L, C, HID), dt.bfloat16, kind="ExternalInput")
    w2_d = nc.dram_tensor("w2", (L, HID, C), dt.bfloat16, kind="ExternalInput")
    hwT_d = nc.dram_tensor("hwT", (TD, C, V1), dt.bfloat16, kind="ExternalInput")
    out_d = nc.dram_tensor("logits", (TOK, V1), dt.float32, kind="ExternalOutput")

    RG = [[0, 1], [2, 3], [4, 5], [6, 7]]
    KVHALF = C * TOK  # elements per k (or v) shard

    with tile.TileContext(nc) as tc:
        consts = tc.alloc_tile_pool(name="consts", bufs=1)
        hpool = tc.alloc_tile_pool(name="hpool", bufs=1)
        wpool = tc.alloc_tile_pool(name="wpool", bufs=1)
        w1pool = tc.alloc_tile_pool(name="w1pool", bufs=2)
        w2pool = tc.alloc_tile_pool(name="w2pool", bufs=4)
        apool = tc.alloc_tile_pool(name="apool", bufs=2)
        dpool = tc.alloc_tile_pool(name="dpool", bufs=2, space="DRAM")
        ps = tc.alloc_tile_pool(name="ps", bufs=8, space="PSUM")

        # ---- constants ----
        mask_sb = consts.tile([128, KSUB, TOK], dt.bfloat16, name="mask_sb")
        nc.sync.dma_start(mask_sb, maskT_d.rearrange("s p q -> p s q"))
        pb_sb = consts.tile([64, NH, 64], dt.float32, name="pb_sb")
        nc.sync.dma_start(pb_sb, pbT_d[:])
        ones32 = consts.tile([128, 1], dt.float32, name="ones32")
        nc.vector.memset(ones32, 1.0)
        ones16 = consts.tile([128, 1], dt.bfloat16, name="ones16")
        nc.vector.memset(ones16, 1.0)
        eps_sb = consts.tile([1, 1], dt.float32, name="eps_sb")
        nc.vector.memset(eps_sb, EPS)

        # ---- residual stream, feature-major fp32 ----
        h0T_r = h0T_d.rearrange("(ko p) q -> p ko q", p=128)
        h_sb = hpool.tile([128, KSUB, TOK], dt.float32, name="h_sb")
        for ct in range(KSUB):
            nc.sync.dma_start(h_sb[:, ct, :], h0T_r[:, ct, :])

        def layernorm(dst_bf16):
            """LN over C (partition axis x KSUB tiles) of h_sb -> dst (bf16)."""
            sum_ps = ps.tile([1, TOK], dt.float32, tag="ps", name="sum_ps")
            ssq_ps = ps.tile([1, TOK], dt.float32, tag="ps", name="ssq_ps")
            for ct in range(KSUB):
                nc.tensor.matmul(
                    sum_ps, ones32, h_sb[:, ct, :],
                    start=(ct == 0), stop=(ct == KSUB - 1),
                )
            for ct in range(KSUB):
                sq = apool.tile([128, TOK], dt.float32, tag="sq", name="sq")
                nc.vector.tensor_mul(sq, h_sb[:, ct, :], h_sb[:, ct, :])
                nc.tensor.matmul(
                    ssq_ps, ones32, sq,
                    start=(ct == 0), stop=(ct == KSUB - 1),
                )
            # stats on one partition
            stat2 = apool.tile([1, 2 * TOK], dt.float32, tag="stat2", name="stat2")
            mean = stat2[:, 0:TOK]
            rstd = stat2[:, TOK : 2 * TOK]
            nc.vector.tensor_scalar_mul(mean, sum_ps, 1.0 / C)
            nc.vector.tensor_scalar_mul(rstd, ssq_ps, 1.0 / C)
            msq = apool.tile([1, TOK], dt.float32, tag="msq", name="msq")
            nc.vector.tensor_mul(msq, mean, mean)
            nc.vector.tensor_sub(rstd, rstd, msq)  # var
            nc.scalar.activation(rstd, rstd, AF.Sqrt, bias=eps_sb, scale=1.0)
            nc.vector.reciprocal(rstd, rstd)
            # broadcast (1, 2*TOK) -> (128, 2*TOK) via DRAM round-trip
            stat_dram = dpool.tile([1, 2 * TOK], dt.float32, tag="statd", name="stat_dram")
            nc.sync.dma_start(stat_dram, stat2)
            statB = apool.tile([128, 2 * TOK], dt.float32, tag="statB", name="statB")
            nc.sync.dma_start(statB, stat_dram.to_broadcast([128, 2 * TOK]))
            meanB = statB[:, 0:TOK]
            rstdB = statB[:, TOK : 2 * TOK]
            for ct in range(KSUB):
                tmp = apool.tile([128, TOK], dt.float32, tag="lntmp", name="lntmp")
                nc.vector.tensor_sub(tmp, h_sb[:, ct, :], meanB)
                nc.vector.tensor_mul(dst_bf16[:, ct, :], tmp, rstdB)

        for layer in range(L):
            # ---------- LN1 ----------
            aT = apool.tile([128, KSUB, TOK], dt.bfloat16, tag="aT", name="aT")
            layernorm(aT)

            # ---------- K, V projections + AllGather ----------
            kv_in = dpool.tile([2, KVHALF], dt.bfloat16, tag="kvin", name="kv_in")
            kv_ga = dpool.tile([2, 2, KVHALF], dt.bfloat16, tag="kvga", name="kv_ga")
            kin = kv_in[0].rearrange("(c t) -> c t", t=TOK)  # (1024, 512) head-major
            vin = kv_in[1].rearrange("(t c) -> t c", c=C)  # (512, 1024) token-major

            wk_sb = wpool.tile([128, KSUB, C], dt.bfloat16, tag="wmat", name="wk_sb")
            nc.sync.dma_start(wk_sb, wk_d[layer].rearrange("(ko p) n -> p ko n", p=128))
            for hh in range(NH):
                pk = ps.tile([128, TOK], dt.float32, tag="ps", name="pk")
                for ct in range(KSUB):
                    nc.tensor.matmul(
                        pk, wk_sb[:, ct, hh * HD : (hh + 1) * HD], aT[:, ct, :],
                        start=(ct == 0), stop=(ct == KSUB - 1),
                    )
                k_sb = apool.tile([128, TOK], dt.bfloat16, tag="kv_out", name="k_sb")
                nc.vector.tensor_copy(k_sb, pk)
                nc.sync.dma_start(kin[hh * HD : (hh + 1) * HD, :], k_sb)

            wv_sb = wpool.tile([128, KSUB, C], dt.bfloat16, tag="wmat", name="wv_sb")
            nc.sync.dma_start(wv_sb, wv_d[layer].rearrange("(ko p) n -> p ko n", p=128))
            for tsub in range(TOK // 128):
                for chalf in range(2):
                    pv = ps.tile([128, 512], dt.float32, tag="ps", name="pv")
                    for ct in range(KSUB):
                        nc.tensor.matmul(
                            pv,
                            aT[:, ct, tsub * 128 : (tsub + 1) * 128],
                            wv_sb[:, ct, chalf * 512 : (chalf + 1) * 512],
                            start=(ct == 0), stop=(ct == KSUB - 1),
                        )
                    v_sb = apool.tile([128, 512], dt.bfloat16, tag="kv_out", name="v_sb")
                    nc.vector.tensor_copy(v_sb, pv)
                    nc.sync.dma_start(
                        vin[tsub * 128 : (tsub + 1) * 128, chalf * 512 : (chalf + 1) * 512],
                        v_sb,
                    )

            nc.gpsimd.collective_compute(
                "AllGather",
                OP.bypass,
                replica_groups=RG,
                ins=[kv_in.opt()],
                outs=[kv_ga.opt()],
            )

            # ---------- Q projection (overlaps AG) ----------
            wq_sb = wpool.tile([128, KSUB, C], dt.bfloat16, tag="wmat", name="wq_sb")
            nc.sync.dma_start(wq_sb, wq_d[layer].rearrange("(ko p) n -> p ko n", p=128))
            qT = apool.tile([128, NH, TOK], dt.bfloat16, tag="qT", name="qT", bufs=1)
            for hh in range(NH):
                pq = ps.tile([128, TOK], dt.float32, tag="ps", name="pq")
                for ct in range(KSUB):
                    nc.tensor.matmul(
                        pq, wq_sb[:, ct, hh * HD : (hh + 1) * HD], aT[:, ct, :],
                        start=(ct == 0), stop=(ct == KSUB - 1),
                    )
                nc.vector.tensor_copy(qT[:, hh, :], pq)

            # ---------- attention ----------
            yT = apool.tile([128, NH, TOK], dt.bfloat16, tag="yT", name="yT", bufs=1)
            for hh in range(NH):
                kg = apool.tile([128, T], dt.bfloat16, tag="kg", name="kg")
                vg = apool.tile([128, KSUB, HD], dt.bfloat16, tag="vg", name="vg")
                for r in range(2):
                    kga = kv_ga[r, 0].rearrange("(c t) -> c t", t=TOK)
                    vga = kv_ga[r, 1].rearrange("(t c) -> t c", c=C)
                    nc.sync.dma_start(
                        kg[:, r * TOK : (r + 1) * TOK],
                        kga[hh * HD : (hh + 1) * HD, :],
                    )
                    for tsub in range(TOK // 128):
                        nc.sync.dma_start(
                            vg[:, r * (TOK // 128) + tsub, :],
                            vga[tsub * 128 : (tsub + 1) * 128, hh * HD : (hh + 1) * HD],
                        )

                e_sb = apool.tile([128, KSUB, TOK], dt.bfloat16, tag="e_sb", name="e_sb")
                den_ps = ps.tile([1, TOK], dt.float32, tag="ps", name="den_ps")
                py = ps.tile([128, TOK], dt.float32, tag="ps", name="py")
                for g in range(KSUB):  # kv subblocks over gathered T
                    s_ps = ps.tile([128, TOK], dt.float32, tag="ps", name="s_ps")
                    nc.tensor.matmul(
                        s_ps, kg[:, g * 128 : (g + 1) * 128], qT[:, hh, :],
                        start=True, stop=True,
                    )
                    if g == 0:
                        nc.vector.tensor_add(
                            s_ps[0:64, 0:64], s_ps[0:64, 0:64], pb_sb[:, hh, :]
                        )
                    ex = apool.tile([128, TOK], dt.float32, tag="ex", name="ex")
                    nc.scalar.activation(ex, s_ps, AF.Exp)
                    nc.vector.tensor_mul(e_sb[:, g, :], ex, mask_sb[:, g, :])
                    nc.tensor.matmul(
                        den_ps, ones16, e_sb[:, g, :],
                        start=(g == 0), stop=(g == KSUB - 1),
                    )
                    nc.tensor.matmul(
                        py, vg[:, g, :], e_sb[:, g, :],
                        start=(g == 0), stop=(g == KSUB - 1),
                    )
                rec = apool.tile([1, TOK], dt.float32, tag="rec", name="rec")
                nc.vector.reciprocal(rec, den_ps)
                rec_dram = dpool.tile([1, TOK], dt.float32, tag="recd", name="rec_dram")
                nc.sync.dma_start(rec_dram, rec)
                recB = apool.tile([128, TOK], dt.float32, tag="recB", name="recB")
                nc.sync.dma_start(recB, rec_dram.to_broadcast([128, TOK]))
                nc.vector.tensor_mul(yT[:, hh, :], py, recB)

            # ---------- proj + residual ----------
            wp_sb = wpool.tile([128, KSUB, C], dt.bfloat16, tag="wmat", name="wp_sb")
            nc.sync.dma_start(wp_sb, wp_d[layer].rearrange("(ko p) n -> p ko n", p=128))
            for co in range(KSUB):
                pp = ps.tile([128, TOK], dt.float32, tag="ps", name="pp")
                for ct in range(KSUB):
                    nc.tensor.matmul(
                        pp, wp_sb[:, ct, co * 128 : (co + 1) * 128], yT[:, ct, :],
                        start=(ct == 0), stop=(ct == KSUB - 1),
                    )
                nc.vector.tensor_add(h_sb[:, co, :], h_sb[:, co, :], pp)

            # ---------- LN2 ----------
            a2T = apool.tile([128, KSUB, TOK], dt.bfloat16, tag="aT", name="a2T")
            layernorm(a2T)

            # ---------- MLP ----------
            g_sb = apool.tile([128, HSUB, TOK], dt.bfloat16, tag="g_sb", name="g_sb", bufs=1)
            for hblk in range(8):  # 512 hidden cols at a time
                w1_sb = w1pool.tile([128, KSUB, 512], dt.bfloat16, tag="w1b", name="w1_sb")
                nc.sync.dma_start(
                    w1_sb,
                    w1_d[layer].rearrange("(ko p) n -> p ko n", p=128)[
                        :, :, hblk * 512 : (hblk + 1) * 512
                    ],
                )
                for hc in range(4):
                    pu = ps.tile([128, TOK], dt.float32, tag="ps", name="pu")
                    for ct in range(KSUB):
                        nc.tensor.matmul(
                            pu,
                            w1_sb[:, ct, hc * 128 : (hc + 1) * 128],
                            a2T[:, ct, :],
                            start=(ct == 0), stop=(ct == KSUB - 1),
                        )
                    nc.scalar.activation(g_sb[:, hblk * 4 + hc, :], pu, AF.Gelu)

            for grp in range(2):  # 4 output c-tiles at a time (PSUM budget)
                pd = [
                    ps.tile([128, TOK], dt.float32, tag="ps", name=f"pd{i}")
                    for i in range(4)
                ]
                for ksub in range(HSUB):
                    w2_sb = w2pool.tile([128, C], dt.bfloat16, tag="w2t", name="w2_sb")
                    nc.sync.dma_start(
                        w2_sb,
                        w2_d[layer].rearrange("(ko p) n -> p ko n", p=128)[:, ksub, :],
                    )
                    for i in range(4):
                        co = grp * 4 + i
                        nc.tensor.matmul(
                            pd[i],
                            w2_sb[:, co * 128 : (co + 1) * 128],
                            g_sb[:, ksub, :],
                            start=(ksub == 0), stop=(ksub == HSUB - 1),
                        )
                for i in range(4):
                    co = grp * 4 + i
                    nc.vector.tensor_add(h_sb[:, co, :], h_sb[:, co, :], pd[i])

        # ---------- final LN + head ----------
        hfT = apool.tile([128, KSUB, TOK], dt.bfloat16, tag="aT", name="hfT")
        layernorm(hfT)
        hfT_r = hfT.rearrange("p k (b e) -> p k e b", e=TD)  # b: 8 blocks of 64
        out_r = out_d.rearrange("(b e) v -> e b v", e=TD)
        for e in range(TD):
            hw_sb = w1pool.tile([128, KSUB, V1], dt.bfloat16, tag="hw", name="hw_sb")
            nc.sync.dma_start(hw_sb, hwT_d[e].rearrange("(ko p) n -> p ko n", p=128))
            po = ps.tile([TOK // TD, V1], dt.float32, tag="ps", name="po")
            for ct in range(KSUB):
                nc.tensor.matmul(
                    po, hfT_r[:, ct, e, :], hw_sb[:, ct, :],
                    start=(ct == 0), stop=(ct == KSUB - 1),
                )
            o_sb = apool.tile([TOK // TD, V1], dt.float32, tag="o_sb", name="o_sb")
            nc.vector.tensor_copy(o_sb, po)
            nc.sync.dma_start(out_r[e], o_sb)

        for p in (ps, dpool, apool, w2pool, w1pool, wpool, hpool, consts):
            p.release()

    nc.compile()
    return nc


def _host_inputs(x, attn_bias, pos_emb, Wq, Wk, Wv, Wp, w1, w2, head_w):
    """Build per-core input maps (numpy)."""
    scale = 1.0 / np.sqrt(HD)
    wq = (np.asarray(Wq, np.float32) * scale).astype(BF16)
    wk = np.asarray(Wk, np.float32).astype(BF16)
    wv = np.asarray(Wv, np.float32).astype(BF16)
    wp = np.asarray(Wp, np.float32).astype(BF16)
    w1b = np.asarray(w1, np.float32).astype(BF16)
    w2b = np.asarray(w2, np.float32).astype(BF16)
    hwT = np.ascontiguousarray(
        np.asarray(head_w, np.float32).transpose(0, 2, 1)
    ).astype(BF16)

    # causal + column-disable mask, transposed per core: maskT[j, i_local]
    i_idx = np.arange(T)
    col_ok = (i_idx % TD) != TD - 1
    M = (i_idx[None, :] <= i_idx[:, None]) & col_ok[None, :]  # M[i, j]

    # pbias (graph bias) expanded; transposed (kv, head, q), padded 60->64
    bias = np.repeat(np.repeat(np.asarray(attn_bias, np.float32), F_DIM, 1), F_DIM, 2)
    pbT = np.zeros((64, NH, 64), np.float32)
    pbT[:D_BIAS, :, :D_BIAS] = bias.transpose(2, 0, 1)  # [j, h, i]
    pbT_zero = np.zeros_like(pbT)

    h0 = np.asarray(x, np.float32) + np.asarray(pos_emb, np.float32)  # (B, T, C)

    in_maps = []
    for core in range(NCORES):
        b, half = core // 2, core % 2
        rows = slice(half * TOK, (half + 1) * TOK)
        h0T = np.ascontiguousarray(h0[b, rows].T)  # (C, TOK)
        maskT = np.ascontiguousarray(M[rows].T).astype(BF16)  # (T, TOK)
        in_maps.append(
            {
                "h0T": h0T,
                "maskT": maskT.reshape(KSUB, 128, TOK),
                "pbT": pbT if half == 0 else pbT_zero,
                "wq": wq, "wk": wk, "wv": wv, "wp": wp,
                "w1": w1b, "w2": w2b, "hwT": hwT,
            }
        )
    return in_maps


def kernel(**inputs):
    from concourse.bass_utils import run_bass_kernel_spmd

    in_maps = _host_inputs(
        inputs["x"], inputs["attn_bias"], inputs["pos_emb"],
        inputs["Wq"], inputs["Wk"], inputs["Wv"], inputs["Wp"],
        inputs["w1"], inputs["w2"], inputs["head_w"],
    )
    if "nc" not in _CACHED:
        _CACHED["nc"] = _build_program()
    res = run_bass_kernel_spmd(
        _CACHED["nc"], in_maps, core_ids=list(range(NCORES)),
        trace=bool(int(os.environ.get("KERNEL_TRACE", "0"))),
    )
    out = np.zeros((B, T, V1), np.float32)
    for core in range(NCORES):
        b, half = core // 2, core % 2
        out[b, half * TOK : (half + 1) * TOK] = res.results[core]["logits"]
    _CACHED["last_result"] = res
    return out

